# revision 13
# baseline (speedup 1.0000x reference)
"""Trainium2 Bass kernel for nn_Bilinear_86328842650062.

Computes out[s,i,j] = sum_{d,e} tensor1[s,i,d] * W[d,e] * tensor0[s,j,e] + bias
for S=4, N=4096, D=64, then tiles to batch 2:  output (2, 4, 4096, 4096) f32.

Strategy (classic 1D row-parallel): shard the i axis (rows of tensor1 /
rows of the output) across 8 NeuronCores, 512 rows each; replicate the
small (D,D) kernel and tensor0.  B = tensor1 @ W (0.5% of the FLOPs) is
computed on the HOST in f32; each core receives its B^T shard and the
full tensor0, both pre-transposed so the contraction dim d lands on SBUF
partitions (two s-slices packed per 128 partitions), and runs per s:
    out_shard[s] = B[s] @ x0[s]^T     (512x64 @ 64x4096, fp16 PE tiles)
The batch-2 leading dim is a pure broadcast, materialized host-side as a
stride-0 view; the f16->f32 output upcast is also host-side.

Performance notes (from NTFF profiles):
  * TRN2's PE has a DVFS ramp (0.65 -> 1.2 -> 2.4 GHz after ~3us of
    continuous work).  Without priming, the whole kernel runs matmuls at
    the lowest p-state (756ns per 512-col tile instead of ~213ns).  We
    issue WARMUP dummy matmuls on a zeroed SBUF tile while the x0 input
    DMA is still in flight, so the PE is hot when real work arrives.
  * Output drain (16.8 MB f16 per core) is the roofline: ~50us at the
    ~330-400 GB/s per-core DMA bandwidth.  psum->sbuf casts are split
    ACT/DVE; output DMA issues alternate the sync (HWDGE) and gpsimd
    (SWDGE) queues so the ACT engine never stalls on descriptor setup.
  * fp16 single-pass matmul gives ~4.5e-4 rel error (gate is 2e-2).
"""

import os as _os

import numpy as np

S, N, D = 4, 4096, 64
S2 = S // 2
N_CORES = 8
ROWS = N // N_CORES  # 512 output rows per core
BATCH = 2

WARMUP = int(_os.environ.get("BASS_WARMUP", "26"))  # PE HAM-priming matmuls
WARMCOLS = int(_os.environ.get("BASS_WARMCOLS", "128"))  # cols per priming matmul

_CACHE = {}


def _build(warmup):
    import concourse.bacc as bacc
    import concourse.tile as tile
    import concourse.mybir as mybir

    dt = mybir.dt
    f32 = dt.float32
    f16 = dt.float16

    nc = bacc.Bacc(
        "TRN2",
        target_bir_lowering=False,
        debug=False,
        enable_asserts=False,
        num_devices=N_CORES,
    )
    # bt = host-computed (x1_shard @ W)^T, f16, packed partition-major:
    # bt[64*(s%2)+d, (s//2)*ROWS + i] = B[s, i, d].
    bt_dram = nc.dram_tensor("bt", [128, S2 * ROWS], f16, kind="ExternalInput").ap()
    # x0 = tensor0^T (S, D, N) f16, DMA'd as [128, S2, N] with p = 64*(s%2)+d.
    x0_dram = nc.dram_tensor("x0", [S, D, N], f16, kind="ExternalInput").ap()
    out_dram = nc.dram_tensor("out", [S, ROWS, N], f16, kind="ExternalOutput").ap()

    IT = ROWS // 128  # 4 psum row-tiles per s
    JB = N // 1024    # 4 psum-pair col-blocks per row-tile

    with tile.TileContext(nc) as tc:
        with (
            tc.tile_pool(name="const", bufs=1) as const_pool,
            tc.tile_pool(name="outsb", bufs=4) as out_pool,
            tc.tile_pool(name="pso", bufs=4, space="PSUM") as pso_pool,
        ):
            bt_sb = const_pool.tile([128, S2 * ROWS], f16)
            x0_sb = const_pool.tile([128, S2, N], f16)
            x0_r = x0_dram.rearrange("(a ps) d x -> (ps d) a x", ps=2)
            warm_sb = const_pool.tile([128, 640], f16)

            # Input loads: bt (256 KB) on sync unblocks the first matmul.
            # Each x0 a-slab [128, 4096] f16 is a fully CONTIGUOUS 1 MB DRAM
            # span (partition-major (ps d) matches DRAM row order), so whole-
            # slab DMAs merge into 8 KB packets at full engine rate; column
            # chunking would shatter them into half-rate 4 KB packets.
            nc.sync.dma_start(bt_sb[:], bt_dram[:])
            # a=0 slab split at the partition boundary: both halves stay
            # contiguous in DRAM, and two queues land it ~2us sooner.
            nc.scalar.dma_start(x0_sb[0:64, 0, :], x0_r[0:64, 0, :])
            nc.gpsimd.dma_start(x0_sb[64:128, 0, :], x0_r[64:128, 0, :])
            nc.gpsimd.dma_start(x0_sb[:, 1, :], x0_r[:, 1, :])

            # PE HAM priming: dummy matmuls with no input dependencies carry
            # the PE through the ~3.4us busy window that releases the clock
            # gate (K=4/8 -> 8/8) while inputs stream in.  The dummy ACT/DVE
            # copies pull the lazy ACT_TABLE_LOAD (~1.3us) and engine spin-up
            # out of the critical path so the first real psum tiles don't
            # backpressure the PE (an idle PE re-throttles the clock).
            if warmup:
                nc.vector.memset(warm_sb[:], 0.0)
                scratch = const_pool.tile([128, 64], f16)
                nc.scalar.copy(scratch[:, 0:32], warm_sb[:, 0:32])
                nc.vector.tensor_copy(scratch[:, 32:64], warm_sb[:, 32:64])
                # Short matmuls keep the PE continuously busy (what the HAM
                # busy-window wants) while burning ~4x fewer PE cycles of the
                # 50%-utilization budget than full 512-col dummies would.
                for _ in range(warmup):
                    ps_w = pso_pool.tile([128, 1024], f32, tag="ps")
                    nc.tensor.matmul(
                        ps_w[:, 0:WARMCOLS],
                        warm_sb[:, 0:128],
                        warm_sb[:, 128 : 128 + WARMCOLS],
                        start=True,
                        stop=True,
                    )

            copy_rt = 0
            for a in range(S2):
                for h in range(2):
                    s = 2 * a + h
                    p0 = h * D
                    for it in range(IT):
                        out_sb = out_pool.tile([128, N], f16)
                        stat = bt_sb[p0 : p0 + D, a * ROWS + it * 128 : a * ROWS + (it + 1) * 128]
                        isl = slice(it * 128, (it + 1) * 128)
                        for jb in range(JB):
                            ps_o = pso_pool.tile([128, 1024], f32, tag="ps")
                            for hh in range(2):
                                j0 = jb * 1024 + hh * 512
                                nc.tensor.matmul(
                                    ps_o[:, hh * 512 : (hh + 1) * 512],
                                    stat,
                                    x0_sb[p0 : p0 + D, a, j0 : j0 + 512],
                                    start=True,
                                    stop=True,
                                )
                            dst = out_sb[:, jb * 1024 : (jb + 1) * 1024]
                            # ACT (1.2 GHz) outpaces DVE (0.96 GHz) on psum
                            # reads; steal every 4th row-tile's 4th block from
                            # DVE for a ~9:7 split that balances both engines.
                            last_rt = copy_rt == S * IT - 1
                            if last_rt:
                                # Final row-tile: halve the drain tail by
                                # splitting each copy across both engines.
                                nc.scalar.copy(dst[:, 0:512], ps_o[:, 0:512])
                                nc.vector.tensor_copy(dst[:, 512:1024], ps_o[:, 512:1024])
                            else:
                                act_copy = jb % 2 == 0 or (jb == 3 and copy_rt % 4 == 3)
                                if act_copy:
                                    nc.scalar.copy(dst, ps_o[:])
                                else:
                                    nc.vector.tensor_copy(dst, ps_o[:])
                            # Drain every finished 1024-col block right away,
                            # strictly alternating the sync (HWDGE) and gpsimd
                            # (SWDGE) rings so neither queue builds a backlog;
                            # ACT stays copy-only.
                            eng = nc.sync if jb % 2 == 0 else nc.gpsimd
                            nsl = slice(jb * 1024, (jb + 1) * 1024)
                            eng.dma_start(out_dram[s, isl, nsl], out_sb[:, nsl])
                        copy_rt += 1
    nc.compile()
    return nc


def _get_nc():
    key = WARMUP
    if key not in _CACHE:
        _CACHE[key] = _build(WARMUP)
    return _CACHE[key]


LAST_RESULTS = None


def kernel(**inputs):
    from concourse.bass_utils import run_bass_kernel_spmd

    global LAST_RESULTS

    tensor0 = np.ascontiguousarray(np.asarray(inputs["tensor0"], dtype=np.float32))
    tensor1 = np.ascontiguousarray(np.asarray(inputs["tensor1"], dtype=np.float32))
    W = np.ascontiguousarray(np.asarray(inputs["kernel"], dtype=np.float32))
    bias = float(np.asarray(inputs["bias"]))

    # Host prep: B = x1 @ W in f32 (0.5% of total FLOPs), then transpose both
    # operands so the contraction dim d is partition-major, and cast to f16.
    x0t = np.ascontiguousarray(tensor0.transpose(0, 2, 1)).astype(np.float16)  # (S,D,N)
    Bt = (tensor1 @ W).transpose(0, 2, 1).astype(np.float16)  # (S, D, N) f16

    in_maps = []
    for c in range(N_CORES):
        # Pack B^T shard partition-major: bt[64*(s%2)+d, (s//2)*ROWS+i].
        bc = Bt[:, :, c * ROWS : (c + 1) * ROWS]  # (S, D, ROWS)
        bt = np.empty((128, S2 * ROWS), dtype=np.float16)
        for a in range(S2):
            csl = slice(a * ROWS, (a + 1) * ROWS)
            bt[0:D, csl] = bc[2 * a]
            bt[D : 2 * D, csl] = bc[2 * a + 1]
        in_maps.append({"bt": bt, "x0": x0t})

    nc = _get_nc()
    res = run_bass_kernel_spmd(nc, in_maps, list(range(N_CORES)))
    LAST_RESULTS = res

    out_full = np.empty((S, N, N), dtype=np.float32)
    for c in range(N_CORES):
        out_full[:, c * ROWS : (c + 1) * ROWS, :] = res.results[c]["out"].astype(
            np.float32, copy=False
        )

    if bias != 0.0:
        out_full += np.float32(bias)

    return np.broadcast_to(out_full[None], (BATCH, S, N, N))


# revision 14
# speedup vs baseline: 1.0317x; 1.0317x over previous
"""Trainium2 Bass kernel for nn_Bilinear_86328842650062.

Computes out[s,i,j] = sum_{d,e} tensor1[s,i,d] * W[d,e] * tensor0[s,j,e] + bias
for S=4, N=4096, D=64, then tiles to batch 2:  output (2, 4, 4096, 4096) f32.

Strategy (classic 1D row-parallel): shard the i axis (rows of tensor1 /
rows of the output) across 8 NeuronCores, 512 rows each; replicate the
small (D,D) kernel and tensor0.  B = tensor1 @ W (0.5% of the FLOPs) is
computed on the HOST in f32; each core receives its B^T shard and the
full tensor0, both pre-transposed so the contraction dim d lands on SBUF
partitions (two s-slices packed per 128 partitions), and runs per s:
    out_shard[s] = B[s] @ x0[s]^T     (512x64 @ 64x4096, fp16 PE tiles)
The batch-2 leading dim is a pure broadcast, materialized host-side as a
stride-0 view; the f16->f32 output upcast is also host-side.

Performance notes (from NTFF profiles):
  * TRN2's PE has a DVFS ramp (0.65 -> 1.2 -> 2.4 GHz after ~3us of
    continuous work).  Without priming, the whole kernel runs matmuls at
    the lowest p-state (756ns per 512-col tile instead of ~213ns).  We
    issue WARMUP dummy matmuls on a zeroed SBUF tile while the x0 input
    DMA is still in flight, so the PE is hot when real work arrives.
  * Output drain (16.8 MB f16 per core) is the roofline: ~50us at the
    ~330-400 GB/s per-core DMA bandwidth.  psum->sbuf casts are split
    ACT/DVE; output DMA issues alternate the sync (HWDGE) and gpsimd
    (SWDGE) queues so the ACT engine never stalls on descriptor setup.
  * fp16 single-pass matmul gives ~4.5e-4 rel error (gate is 2e-2).
"""

import os as _os

import numpy as np

S, N, D = 4, 4096, 64
S2 = S // 2
N_CORES = 8
ROWS = N // N_CORES  # 512 output rows per core
BATCH = 2

WARMUP = int(_os.environ.get("BASS_WARMUP", "12"))  # PE HAM-priming matmuls
WARMCOLS = int(_os.environ.get("BASS_WARMCOLS", "512"))  # cols per priming matmul

_CACHE = {}


def _build(warmup):
    import concourse.bacc as bacc
    import concourse.tile as tile
    import concourse.mybir as mybir

    dt = mybir.dt
    f32 = dt.float32
    f16 = dt.float16

    nc = bacc.Bacc(
        "TRN2",
        target_bir_lowering=False,
        debug=False,
        enable_asserts=False,
        num_devices=N_CORES,
    )
    # bt = host-computed (x1_shard @ W)^T, f16, packed partition-major:
    # bt[64*(s%2)+d, (s//2)*ROWS + i] = B[s, i, d].
    bt_dram = nc.dram_tensor("bt", [128, S2 * ROWS], f16, kind="ExternalInput").ap()
    # x0 = tensor0^T (S, D, N) f16, DMA'd as [128, S2, N] with p = 64*(s%2)+d.
    x0_dram = nc.dram_tensor("x0", [S, D, N], f16, kind="ExternalInput").ap()
    out_dram = nc.dram_tensor("out", [S, ROWS, N], f16, kind="ExternalOutput").ap()

    IT = ROWS // 128  # 4 psum row-tiles per s
    JB = N // 1024    # 4 psum-pair col-blocks per row-tile

    with tile.TileContext(nc) as tc:
        with (
            tc.tile_pool(name="const", bufs=1) as const_pool,
            tc.tile_pool(name="outsb", bufs=4) as out_pool,
            tc.tile_pool(name="pso", bufs=4, space="PSUM") as pso_pool,
        ):
            bt_sb = const_pool.tile([128, S2 * ROWS], f16)
            x0_sb = const_pool.tile([128, S2, N], f16)
            x0_r = x0_dram.rearrange("(a ps) d x -> (ps d) a x", ps=2)
            warm_sb = const_pool.tile([128, 640], f16)

            # Input loads: bt (256 KB) on sync unblocks the first matmul.
            # Each x0 a-slab [128, 4096] f16 is a fully CONTIGUOUS 1 MB DRAM
            # span (partition-major (ps d) matches DRAM row order), so whole-
            # slab DMAs merge into 8 KB packets at full engine rate; column
            # chunking would shatter them into half-rate 4 KB packets.
            nc.sync.dma_start(bt_sb[:], bt_dram[:])
            # a=0 slab split at the partition boundary: both halves stay
            # contiguous in DRAM, and two queues land it ~2us sooner.
            nc.scalar.dma_start(x0_sb[0:64, 0, :], x0_r[0:64, 0, :])
            nc.gpsimd.dma_start(x0_sb[64:128, 0, :], x0_r[64:128, 0, :])
            nc.gpsimd.dma_start(x0_sb[:, 1, :], x0_r[:, 1, :])

            # PE HAM priming: dummy matmuls with no input dependencies carry
            # the PE through the ~3.4us busy window that releases the clock
            # gate (K=4/8 -> 8/8) while inputs stream in.  The dummy ACT/DVE
            # copies pull the lazy ACT_TABLE_LOAD (~1.3us) and engine spin-up
            # out of the critical path so the first real psum tiles don't
            # backpressure the PE (an idle PE re-throttles the clock).
            if warmup:
                nc.vector.memset(warm_sb[:], 0.0)
                scratch = const_pool.tile([128, 64], f16)
                nc.scalar.copy(scratch[:, 0:32], warm_sb[:, 0:32])
                nc.vector.tensor_copy(scratch[:, 32:64], warm_sb[:, 32:64])
                # Short matmuls keep the PE continuously busy (what the HAM
                # busy-window wants) while burning ~4x fewer PE cycles of the
                # 50%-utilization budget than full 512-col dummies would.
                for _ in range(warmup):
                    ps_w = pso_pool.tile([128, 1024], f32, tag="ps")
                    nc.tensor.matmul(
                        ps_w[:, 0:WARMCOLS],
                        warm_sb[:, 0:128],
                        warm_sb[:, 128 : 128 + WARMCOLS],
                        start=True,
                        stop=True,
                    )

            copy_rt = 0
            for a in range(S2):
                for h in range(2):
                    s = 2 * a + h
                    p0 = h * D
                    for it in range(IT):
                        out_sb = out_pool.tile([128, N], f16)
                        stat = bt_sb[p0 : p0 + D, a * ROWS + it * 128 : a * ROWS + (it + 1) * 128]
                        isl = slice(it * 128, (it + 1) * 128)
                        for jb in range(JB):
                            ps_o = pso_pool.tile([128, 1024], f32, tag="ps")
                            for hh in range(2):
                                j0 = jb * 1024 + hh * 512
                                nc.tensor.matmul(
                                    ps_o[:, hh * 512 : (hh + 1) * 512],
                                    stat,
                                    x0_sb[p0 : p0 + D, a, j0 : j0 + 512],
                                    start=True,
                                    stop=True,
                                )
                            dst = out_sb[:, jb * 1024 : (jb + 1) * 1024]
                            # ACT (1.2 GHz) outpaces DVE (0.96 GHz) on psum
                            # reads; steal every 4th row-tile's 4th block from
                            # DVE for a ~9:7 split that balances both engines.
                            last_rt = copy_rt == S * IT - 1
                            if last_rt:
                                # Final row-tile: halve the drain tail by
                                # splitting each copy across both engines.
                                nc.scalar.copy(dst[:, 0:512], ps_o[:, 0:512])
                                nc.vector.tensor_copy(dst[:, 512:1024], ps_o[:, 512:1024])
                            else:
                                act_copy = jb % 2 == 0 or (jb == 3 and copy_rt % 4 == 3)
                                if act_copy:
                                    nc.scalar.copy(dst, ps_o[:])
                                else:
                                    nc.vector.tensor_copy(dst, ps_o[:])
                            # Drain every finished 1024-col block right away,
                            # strictly alternating the sync (HWDGE) and gpsimd
                            # (SWDGE) rings so neither queue builds a backlog;
                            # ACT stays copy-only.
                            eng = nc.sync if jb % 2 == 0 else nc.gpsimd
                            nsl = slice(jb * 1024, (jb + 1) * 1024)
                            eng.dma_start(out_dram[s, isl, nsl], out_sb[:, nsl])
                        copy_rt += 1
    nc.compile()
    return nc


def _get_nc():
    key = WARMUP
    if key not in _CACHE:
        _CACHE[key] = _build(WARMUP)
    return _CACHE[key]


LAST_RESULTS = None


def kernel(**inputs):
    from concourse.bass_utils import run_bass_kernel_spmd

    global LAST_RESULTS

    tensor0 = np.ascontiguousarray(np.asarray(inputs["tensor0"], dtype=np.float32))
    tensor1 = np.ascontiguousarray(np.asarray(inputs["tensor1"], dtype=np.float32))
    W = np.ascontiguousarray(np.asarray(inputs["kernel"], dtype=np.float32))
    bias = float(np.asarray(inputs["bias"]))

    # Host prep: B = x1 @ W in f32 (0.5% of total FLOPs), then transpose both
    # operands so the contraction dim d is partition-major, and cast to f16.
    x0t = np.ascontiguousarray(tensor0.transpose(0, 2, 1)).astype(np.float16)  # (S,D,N)
    Bt = (tensor1 @ W).transpose(0, 2, 1).astype(np.float16)  # (S, D, N) f16

    in_maps = []
    for c in range(N_CORES):
        # Pack B^T shard partition-major: bt[64*(s%2)+d, (s//2)*ROWS+i].
        bc = Bt[:, :, c * ROWS : (c + 1) * ROWS]  # (S, D, ROWS)
        bt = np.empty((128, S2 * ROWS), dtype=np.float16)
        for a in range(S2):
            csl = slice(a * ROWS, (a + 1) * ROWS)
            bt[0:D, csl] = bc[2 * a]
            bt[D : 2 * D, csl] = bc[2 * a + 1]
        in_maps.append({"bt": bt, "x0": x0t})

    nc = _get_nc()
    res = run_bass_kernel_spmd(nc, in_maps, list(range(N_CORES)))
    LAST_RESULTS = res

    out_full = np.empty((S, N, N), dtype=np.float32)
    for c in range(N_CORES):
        out_full[:, c * ROWS : (c + 1) * ROWS, :] = res.results[c]["out"].astype(
            np.float32, copy=False
        )

    if bias != 0.0:
        out_full += np.float32(bias)

    return np.broadcast_to(out_full[None], (BATCH, S, N, N))


# revision 15
# speedup vs baseline: 1.1442x; 1.1090x over previous
"""Trainium2 Bass kernel for nn_Bilinear_86328842650062.

Computes out[s,i,j] = sum_{d,e} tensor1[s,i,d] * W[d,e] * tensor0[s,j,e] + bias
for S=4, N=4096, D=64, then tiles to batch 2:  output (2, 4, 4096, 4096) f32.

Strategy (classic 1D row-parallel): shard the i axis (rows of tensor1 /
rows of the output) across 8 NeuronCores, 512 rows each; replicate the
small (D,D) kernel and tensor0.  B = tensor1 @ W (0.5% of the FLOPs) is
computed on the HOST in f32; each core receives its B^T shard and the
full tensor0, both pre-transposed so the contraction dim d lands on SBUF
partitions (two s-slices packed per 128 partitions), and runs per s:
    out_shard[s] = B[s] @ x0[s]^T     (512x64 @ 64x4096, fp16 PE tiles)
The batch-2 leading dim is a pure broadcast, materialized host-side as a
stride-0 view; the f16->f32 output upcast is also host-side.

Performance notes (from NTFF profiles):
  * TRN2's PE has a DVFS ramp (0.65 -> 1.2 -> 2.4 GHz after ~3us of
    continuous work).  Without priming, the whole kernel runs matmuls at
    the lowest p-state (756ns per 512-col tile instead of ~213ns).  We
    issue WARMUP dummy matmuls on a zeroed SBUF tile while the x0 input
    DMA is still in flight, so the PE is hot when real work arrives.
  * Output drain (16.8 MB f16 per core) is the roofline: ~50us at the
    ~330-400 GB/s per-core DMA bandwidth.  psum->sbuf casts are split
    ACT/DVE; output DMA issues alternate the sync (HWDGE) and gpsimd
    (SWDGE) queues so the ACT engine never stalls on descriptor setup.
  * fp16 single-pass matmul gives ~4.5e-4 rel error (gate is 2e-2).
"""

import os as _os

import numpy as np

S, N, D = 4, 4096, 64
S2 = S // 2
N_CORES = 8
ROWS = N // N_CORES  # 512 output rows per core
BATCH = 2

WARMUP = int(_os.environ.get("BASS_WARMUP", "12"))  # PE HAM-priming matmuls
WARMCOLS = int(_os.environ.get("BASS_WARMCOLS", "512"))  # cols per priming matmul

_CACHE = {}


def _build(warmup):
    import concourse.bacc as bacc
    import concourse.tile as tile
    import concourse.mybir as mybir

    dt = mybir.dt
    f32 = dt.float32
    f16 = dt.float16

    nc = bacc.Bacc(
        "TRN2",
        target_bir_lowering=False,
        debug=False,
        enable_asserts=False,
        num_devices=N_CORES,
    )
    # bt = host-computed (x1_shard @ W)^T, f16, packed partition-major:
    # bt[64*(s%2)+d, (s//2)*ROWS + i] = B[s, i, d].
    bt_dram = nc.dram_tensor("bt", [128, S2 * ROWS], f16, kind="ExternalInput").ap()
    # x0 = tensor0^T (S, D, N) f16, DMA'd as [128, S2, N] with p = 64*(s%2)+d.
    x0_dram = nc.dram_tensor("x0", [S, D, N], f16, kind="ExternalInput").ap()
    out_dram = nc.dram_tensor("out", [S, ROWS, N], f16, kind="ExternalOutput").ap()

    IT = ROWS // 128  # 4 psum row-tiles per s
    JB = N // 1024    # 4 psum-pair col-blocks per row-tile

    with tile.TileContext(nc) as tc:
        with (
            tc.tile_pool(name="const", bufs=1) as const_pool,
            tc.tile_pool(name="outsb", bufs=4) as out_pool,
            tc.tile_pool(name="pso", bufs=4, space="PSUM") as pso_pool,
        ):
            bt_sb = const_pool.tile([128, S2 * ROWS], f16)
            x0_sb = const_pool.tile([128, S2, N], f16)
            x0_r = x0_dram.rearrange("(a ps) d x -> (ps d) a x", ps=2)
            warm_sb = const_pool.tile([128, 640], f16)

            # Input loads: bt (256 KB) on sync unblocks the first matmul.
            # Each x0 a-slab [128, 4096] f16 is a fully CONTIGUOUS 1 MB DRAM
            # span (partition-major (ps d) matches DRAM row order), so whole-
            # slab DMAs merge into 8 KB packets at full engine rate; column
            # chunking would shatter them into half-rate 4 KB packets.
            nc.sync.dma_start(bt_sb[:], bt_dram[:])
            # a=0 slab split at the partition boundary: both halves stay
            # contiguous in DRAM, and two queues land it ~2us sooner.
            nc.scalar.dma_start(x0_sb[0:64, 0, :], x0_r[0:64, 0, :])
            nc.gpsimd.dma_start(x0_sb[64:128, 0, :], x0_r[64:128, 0, :])
            nc.gpsimd.dma_start(x0_sb[:, 1, :], x0_r[:, 1, :])

            # PE HAM priming: dummy matmuls with no input dependencies carry
            # the PE through the ~3.4us busy window that releases the clock
            # gate (K=4/8 -> 8/8) while inputs stream in.  The dummy ACT/DVE
            # copies pull the lazy ACT_TABLE_LOAD (~1.3us) and engine spin-up
            # out of the critical path so the first real psum tiles don't
            # backpressure the PE (an idle PE re-throttles the clock).
            if warmup:
                nc.vector.memset(warm_sb[:], 0.0)
                scratch = const_pool.tile([128, 64], f16)
                nc.scalar.copy(scratch[:, 0:32], warm_sb[:, 0:32])
                nc.vector.tensor_copy(scratch[:, 32:64], warm_sb[:, 32:64])
                # Short matmuls keep the PE continuously busy (what the HAM
                # busy-window wants) while burning ~4x fewer PE cycles of the
                # 50%-utilization budget than full 512-col dummies would.
                for _ in range(warmup):
                    ps_w = pso_pool.tile([128, 1024], f32, tag="ps")
                    nc.tensor.matmul(
                        ps_w[:, 0:WARMCOLS],
                        warm_sb[:, 0:128],
                        warm_sb[:, 128 : 128 + WARMCOLS],
                        start=True,
                        stop=True,
                    )

            copy_rt = 0
            for a in range(S2):
                for h in range(2):
                    s = 2 * a + h
                    p0 = h * D
                    for it in range(IT):
                        out_sb = out_pool.tile([128, N], f16)
                        stat = bt_sb[p0 : p0 + D, a * ROWS + it * 128 : a * ROWS + (it + 1) * 128]
                        isl = slice(it * 128, (it + 1) * 128)
                        for jb in range(JB):
                            ps_o = pso_pool.tile([128, 1024], f32, tag="ps")
                            for hh in range(2):
                                j0 = jb * 1024 + hh * 512
                                nc.tensor.matmul(
                                    ps_o[:, hh * 512 : (hh + 1) * 512],
                                    stat,
                                    x0_sb[p0 : p0 + D, a, j0 : j0 + 512],
                                    start=True,
                                    stop=True,
                                )
                            dst = out_sb[:, jb * 1024 : (jb + 1) * 1024]
                            # ACT (1.2 GHz) outpaces DVE (0.96 GHz) on psum
                            # reads; steal every 4th row-tile's 4th block from
                            # DVE for a ~9:7 split that balances both engines.
                            last_rt = copy_rt == S * IT - 1
                            if last_rt:
                                # Final row-tile: halve the drain tail by
                                # splitting each copy across both engines and
                                # DMAing every 1024-col block on its own queue.
                                nc.scalar.copy(dst[:, 0:512], ps_o[:, 0:512])
                                nc.vector.tensor_copy(dst[:, 512:1024], ps_o[:, 512:1024])
                                eng = nc.sync if jb % 2 == 0 else nc.gpsimd
                                nsl = slice(jb * 1024, (jb + 1) * 1024)
                                eng.dma_start(out_dram[s, isl, nsl], out_sb[:, nsl])
                            else:
                                act_copy = jb % 2 == 0 or (jb == 3 and copy_rt % 4 == 3)
                                if act_copy:
                                    nc.scalar.copy(dst, ps_o[:])
                                else:
                                    nc.vector.tensor_copy(dst, ps_o[:])
                                if jb % 2 == 1:
                                    # Drain each finished 2048-col half right
                                    # away, alternating the sync (HWDGE) and
                                    # gpsimd (SWDGE) rings; ACT stays copy-only.
                                    jh = jb // 2
                                    eng = nc.sync if (copy_rt + jh) % 2 == 0 else nc.gpsimd
                                    nsl = slice(jh * 2048, (jh + 1) * 2048)
                                    eng.dma_start(out_dram[s, isl, nsl], out_sb[:, nsl])
                        copy_rt += 1
    nc.compile()
    return nc


def _get_nc():
    key = WARMUP
    if key not in _CACHE:
        _CACHE[key] = _build(WARMUP)
    return _CACHE[key]


LAST_RESULTS = None


def kernel(**inputs):
    from concourse.bass_utils import run_bass_kernel_spmd

    global LAST_RESULTS

    tensor0 = np.ascontiguousarray(np.asarray(inputs["tensor0"], dtype=np.float32))
    tensor1 = np.ascontiguousarray(np.asarray(inputs["tensor1"], dtype=np.float32))
    W = np.ascontiguousarray(np.asarray(inputs["kernel"], dtype=np.float32))
    bias = float(np.asarray(inputs["bias"]))

    # Host prep: B = x1 @ W in f32 (0.5% of total FLOPs), then transpose both
    # operands so the contraction dim d is partition-major, and cast to f16.
    x0t = np.ascontiguousarray(tensor0.transpose(0, 2, 1)).astype(np.float16)  # (S,D,N)
    Bt = (tensor1 @ W).transpose(0, 2, 1).astype(np.float16)  # (S, D, N) f16

    in_maps = []
    for c in range(N_CORES):
        # Pack B^T shard partition-major: bt[64*(s%2)+d, (s//2)*ROWS+i].
        bc = Bt[:, :, c * ROWS : (c + 1) * ROWS]  # (S, D, ROWS)
        bt = np.empty((128, S2 * ROWS), dtype=np.float16)
        for a in range(S2):
            csl = slice(a * ROWS, (a + 1) * ROWS)
            bt[0:D, csl] = bc[2 * a]
            bt[D : 2 * D, csl] = bc[2 * a + 1]
        in_maps.append({"bt": bt, "x0": x0t})

    nc = _get_nc()
    res = run_bass_kernel_spmd(nc, in_maps, list(range(N_CORES)))
    LAST_RESULTS = res

    out_full = np.empty((S, N, N), dtype=np.float32)
    for c in range(N_CORES):
        out_full[:, c * ROWS : (c + 1) * ROWS, :] = res.results[c]["out"].astype(
            np.float32, copy=False
        )

    if bias != 0.0:
        out_full += np.float32(bias)

    return np.broadcast_to(out_full[None], (BATCH, S, N, N))


# revision 16
# speedup vs baseline: 1.3474x; 1.1776x over previous
"""Trainium2 Bass kernel for nn_Bilinear_86328842650062 — fp8 DoubleRow + int8 out.

Same sharding as kernel.py (rows of tensor1 across 8 cores, tensor0 + the
small W replicated, B = tensor1 @ W computed host-side).  Two changes that
together break the PE-clock and DMA walls:

1.  The big matmul runs in fp8 e4m3 with perf_mode=DoubleRow, which contracts
    TWO (weight, ifmap) planes per partition per cycle — half the stream
    cycles of fp16.  The 2x128 plane layout computes a compensated hi/lo
    product in ONE stream:
        partitions 0-63 : planes (Bhi, Blo) x (Xhi, Xhi)  -> Bhi@Xhi + Blo@Xhi
        partitions 64-127: planes (Bhi,  0) x (Xlo, Xlo)  -> Bhi@Xlo
    where hi/lo are fp8 round + residual (sim: 1.3e-3 rel err).  The ifmap
    planes are a stride-0 broadcast, so x0 ships at 1 byte/element.
2.  The output is int8 with a per-row scale folded into B on the host
    (rows of B scaled by 126/(||B_row|| * max_j||x0_j||), a Cauchy-Schwarz
    bound, so |psum| <= 126).  Uniform quantization bounds ABSOLUTE error:
    ~0.9e-2 of the global absmax (gate 2e-2) while halving output DMA bytes.
    The device does plain f32->int8 casts; the host multiplies the scales
    back during reassembly.
"""

import os as _os

import numpy as np

S, N, D = 4, 4096, 64
N_CORES = 8
ROWS = N // N_CORES
BATCH = 2
IT = ROWS // 128  # 4 row-tiles of 128 output rows per s
JB = N // 1024    # 4 psum-pair col-blocks per row-tile

WARMUP = int(_os.environ.get("BASS_WARMUP", "10"))

_CACHE = {}


def _build(warmup):
    import concourse.bacc as bacc
    import concourse.tile as tile
    import concourse.mybir as mybir

    dt = mybir.dt
    f32 = dt.float32
    f16 = dt.float16
    f8 = dt.float8e4
    i8 = dt.int8
    DR = mybir.MatmulPerfMode.DoubleRow

    nc = bacc.Bacc(
        "TRN2",
        target_bir_lowering=False,
        debug=False,
        enable_asserts=False,
        num_devices=N_CORES,
    )
    bt_dram = nc.dram_tensor("bt8", [128, S * IT, 2, 128], f8, kind="ExternalInput").ap()
    x0_dram = nc.dram_tensor("x08", [S, 128, N], f8, kind="ExternalInput").ap()
    out_dram = nc.dram_tensor("out", [S, ROWS, N], i8, kind="ExternalOutput").ap()

    with tile.TileContext(nc) as tc:
        with (
            tc.tile_pool(name="const", bufs=1) as const_pool,
            tc.tile_pool(name="outsb", bufs=4) as out_pool,
            tc.tile_pool(name="pso", bufs=4, space="PSUM") as pso_pool,
        ):
            bt_sb = const_pool.tile([128, S * IT, 2, 128], f8)
            x0_sb = const_pool.tile([128, S, N], f8)
            warm_sb = const_pool.tile([128, 640], f16)

            # s=0 gets the scalar queue to itself so the first real matmul
            # isn't starved by the other slabs sharing the 16 DMA engines.
            nc.sync.dma_start(bt_sb[:], bt_dram[:])
            nc.scalar.dma_start(x0_sb[:, 0, :], x0_dram[0])
            for s in range(1, S):
                nc.gpsimd.dma_start(x0_sb[:, s, :], x0_dram[s])

            if warmup:
                nc.vector.memset(warm_sb[:], 0.0)
                scratch = const_pool.tile([128, 64], f16)
                nc.scalar.copy(scratch[:, 0:32], warm_sb[:, 0:32])
                nc.vector.tensor_copy(scratch[:, 32:64], warm_sb[:, 32:64])
                for _ in range(warmup):
                    ps_w = pso_pool.tile([128, 1024], f32, tag="ps")
                    nc.tensor.matmul(
                        ps_w[:, 0:512],
                        warm_sb[:, 0:128],
                        warm_sb[:, 128:640],
                        start=True,
                        stop=True,
                    )

            copy_rt = 0
            for s in range(S):
                for it in range(IT):
                    out_sb = out_pool.tile([128, N], i8)
                    stat = bt_sb[:, s * IT + it, :, :]  # [128, 2, 128]
                    isl = slice(it * 128, (it + 1) * 128)
                    last_rt = copy_rt == S * IT - 1
                    for jb in range(JB):
                        ps_o = pso_pool.tile([128, 1024], f32, tag="ps")
                        for hh in range(2):
                            j0 = jb * 1024 + hh * 512
                            mov = (
                                x0_sb[:, s, j0 : j0 + 512]
                                .unsqueeze(1)
                                .broadcast_to([128, 2, 512])
                            )
                            nc.tensor.matmul(
                                ps_o[:, hh * 512 : (hh + 1) * 512],
                                stat,
                                mov,
                                start=True,
                                stop=True,
                                perf_mode=DR,
                            )
                        dst = out_sb[:, jb * 1024 : (jb + 1) * 1024]
                        if last_rt:
                            nc.scalar.copy(dst[:, 0:512], ps_o[:, 0:512])
                            nc.vector.tensor_copy(dst[:, 512:1024], ps_o[:, 512:1024])
                            eng = nc.sync if jb % 2 == 0 else nc.gpsimd
                            nsl = slice(jb * 1024, (jb + 1) * 1024)
                            eng.dma_start(out_dram[s, isl, nsl], out_sb[:, nsl])
                        else:
                            # ~33:31 ACT:DVE balances the engines' int8 rates.
                            act_copy = jb % 2 == 0 or (jb == 3 and copy_rt == 7)
                            if act_copy:
                                nc.scalar.copy(dst, ps_o[:])
                            else:
                                nc.vector.tensor_copy(dst, ps_o[:])
                            if jb == JB - 1:
                                # int8 rows are 4 KB: only the full row-tile is
                                # DRAM-contiguous enough for 8 KB packets, so
                                # drain once per row-tile, alternating queues.
                                eng = nc.sync if copy_rt % 2 == 0 else nc.gpsimd
                                eng.dma_start(out_dram[s, isl, :], out_sb[:])
                    copy_rt += 1
    nc.compile()
    return nc


def _get_nc():
    key = WARMUP
    if key not in _CACHE:
        _CACHE[key] = _build(WARMUP)
    return _CACHE[key]


LAST_RESULTS = None


def kernel(**inputs):
    import ml_dtypes
    from concourse.bass_utils import run_bass_kernel_spmd

    global LAST_RESULTS
    E4 = ml_dtypes.float8_e4m3  # TRN FP8_EXP4 variant (max +-240)

    tensor0 = np.ascontiguousarray(np.asarray(inputs["tensor0"], dtype=np.float32))
    tensor1 = np.ascontiguousarray(np.asarray(inputs["tensor1"], dtype=np.float32))
    W = np.ascontiguousarray(np.asarray(inputs["kernel"], dtype=np.float32))
    bias = float(np.asarray(inputs["bias"]))

    B = tensor1 @ W  # (S,N,D) f32
    x0t = np.ascontiguousarray(tensor0.transpose(0, 2, 1))  # (S,D,N)

    # Per-row Cauchy-Schwarz bound folded into B so psum lands in [-126,126].
    xn = np.linalg.norm(tensor0, axis=2).max(axis=1)  # (S,)
    bn = np.linalg.norm(B, axis=2)  # (S,N)
    rb = np.maximum(bn * xn[:, None], 1e-20)  # (S,N)
    Bs = B * (126.0 / rb)[:, :, None]

    Bhi = Bs.astype(E4)
    Blo = (Bs - Bhi.astype(np.float32)).astype(E4)
    Xhi = x0t.astype(E4)
    Xlo = (x0t - Xhi.astype(np.float32)).astype(E4)
    x08 = np.ascontiguousarray(np.concatenate([Xhi, Xlo], axis=1))  # (S,128,N)

    in_maps = []
    for c in range(N_CORES):
        rsl = slice(c * ROWS, (c + 1) * ROWS)
        # bt8[p, s*IT+it, plane, m]: p<64 -> (Bhi, Blo)[d=p]; p>=64 -> (Bhi, 0)[d=p-64]
        hi = Bhi[:, rsl, :].astype(np.float32).reshape(S, IT, 128, D).transpose(3, 0, 1, 2)
        lo = Blo[:, rsl, :].astype(np.float32).reshape(S, IT, 128, D).transpose(3, 0, 1, 2)
        bt8 = np.zeros((128, S, IT, 2, 128), dtype=E4)
        bt8[0:D, :, :, 0, :] = hi.astype(E4)
        bt8[0:D, :, :, 1, :] = lo.astype(E4)
        bt8[D:128, :, :, 0, :] = hi.astype(E4)
        in_maps.append(
            {"bt8": np.ascontiguousarray(bt8.reshape(128, S * IT, 2, 128)), "x08": x08}
        )

    nc = _get_nc()
    res = run_bass_kernel_spmd(nc, in_maps, list(range(N_CORES)))
    LAST_RESULTS = res

    out_full = np.empty((S, N, N), dtype=np.float32)
    for c in range(N_CORES):
        rsl = slice(c * ROWS, (c + 1) * ROWS)
        q = res.results[c]["out"].astype(np.float32, copy=False)  # (S,ROWS,N)
        out_full[:, rsl, :] = q * (rb[:, rsl] / 126.0)[:, :, None]

    if bias != 0.0:
        out_full += np.float32(bias)

    return np.broadcast_to(out_full[None], (BATCH, S, N, N))
